# revision 1
# baseline (speedup 1.0000x reference)
"""DisenGCN Bass kernel for trn2 (8-core SPMD), v3: round-major layout.

Nodes (and their incoming edges) are partitioned across cores by target
node; within a core, nodes are sorted by in-degree and grouped into 128-node
windows. Edges of window w occupy slot (r, v): round r, node-in-window v
(v = partition index). Rounds [0, RA_w) hold edges whose source row is in
the low half of the all-gathered feature table, rounds [RA_w, RA_w+RB_w)
the high half (so the per-layer z re-gather can use int16-indexed
dma_gather on each half). Per-window round counts vary; degree sorting
keeps total padding small. A bf16 mask (1 if slot holds a real edge)
zeroes padding contributions.

Per routing iteration, per window (c and its bf16 shadow cn live in SBUF):
  z tile [128 v, R_w, 128] (bf16 DRAM stream)
  zc = z * bcast_r(cn_window)                      (DVE, step-0 AP)
  p[v, r, k] = reduce_dd zc                        (DVE)
  pn = exp(p) / sum_k exp(p) * mask                (ACT + DVE; |p|<=1)
  ws = z * bcast_dd(pn)                            (DVE)
  c_window += reduce_r ws                          (DVE strided reduce + add)
Once per layer: AllGather of cn -> z re-gather via 2x dma_gather halves.
The host un-permutes the output rows (degree sort) after the run.
"""

import sys

sys.path.insert(0, "/opt/trn_rl_repo")
import numpy as np
import ml_dtypes
from dataclasses import dataclass

from concourse import bass, mybir, bacc
from concourse.tile import TileContext
from concourse.tile_rust import add_dep_helper
from concourse.library_config import mlp as mlp_lib, standard as std_lib

BF16 = ml_dtypes.bfloat16
F32 = mybir.dt.float32
BF = mybir.dt.bfloat16
I16 = mybir.dt.int16


@dataclass
class Cfg:
    ncores: int = 8
    n_nodes: int = 50000
    in_dim: int = 512
    d: int = 128
    k: int = 8
    routit: int = 4
    nlayer: int = 3
    nclass: int = 16
    nodes_pc: int = 0
    nw: int = 0
    ra: list = None               # per-window low-half rounds
    rb: list = None               # per-window high-half rounds
    cb: int = 48                  # z-gather chunk size in blocks
    unroll_layers: bool = False
    unroll_t: bool = False
    single_packet: bool = False
    gpsimd_ws: bool = False

    @property
    def nloc(self):
        return self.nw * 128

    @property
    def nfull(self):
        return self.ncores * self.nloc

    @property
    def alim(self):              # rows reachable by gather pass A (base 0)
        return min(self.nfull, 32768)

    @property
    def b0(self):                # base row of gather pass B
        return max(0, self.nfull - 32768)

    @property
    def dd(self):
        return self.d // self.k


# ---------------------------------------------------------------- host prep

def wrap16(idx):
    """[n] -> [128, n//16] int16: slot j at partition j%16 (replicated 8x),
    col j//16."""
    n = len(idx)
    assert n % 16 == 0
    w = np.asarray(idx, np.int64).reshape(n // 16, 16).T
    assert w.max() < 32768
    return np.tile(w.astype(np.int16), (8, 1))


def prep(cfg: Cfg, feat, src_trg):
    """Degree-sorted round-major layout with balanced A/B assignment.
    Returns (in_maps, perms); perms[c] maps sorted position -> original id."""
    n, c = cfg.n_nodes, cfg.ncores
    assert n % c == 0
    cfg.nodes_pc = n // c
    cfg.nw = (cfg.nodes_pc + 127) // 128
    src = np.asarray(src_trg[0]).astype(np.int64)
    trg = np.asarray(src_trg[1]).astype(np.int64)

    src_core, src_loc = src // cfg.nodes_pc, src % cfg.nodes_pc
    trg_core, trg_loc = trg // cfg.nodes_pc, trg % cfg.nodes_pc

    # per-core degree sort (stable, descending) over ORIGINAL local ids
    perms, spos = [], []
    deg = np.zeros((c, cfg.nodes_pc), np.int64)
    np.add.at(deg, (trg_core, trg_loc), 1)
    for ci in range(c):
        order = np.argsort(-deg[ci], kind="stable")
        pos = np.empty(cfg.nodes_pc, np.int64)
        pos[order] = np.arange(cfg.nodes_pc)
        perms.append(order)
        spos.append(pos)
    spos_all = np.stack(spos)

    src_row = src_core * cfg.nloc + spos_all[src_core, src_loc]
    tpos = spos_all[trg_core, trg_loc]
    ALIM, B0 = cfg.alim, cfg.b0

    # classify: 0 = A-only (< B0), 1 = flexible, 2 = B-only (>= ALIM)
    cls = np.where(src_row < B0, 0, np.where(src_row >= ALIM, 2, 1))

    in_maps = []
    RA_all = np.zeros((c, cfg.nw), np.int64)
    RB_all = np.zeros((c, cfg.nw), np.int64)
    percore = []
    for ci in range(c):
        m = np.nonzero(trg_core == ci)[0]
        # order edges by (node, class) so flexible edges sit between A and B
        key = tpos[m] * 3 + cls[m]
        eorder = m[np.argsort(key, kind="stable")]
        tp = tpos[eorder]
        kl = cls[eorder]
        # per-node counts
        a_n = np.zeros(cfg.nodes_pc, np.int64)
        f_n = np.zeros(cfg.nodes_pc, np.int64)
        b_n = np.zeros(cfg.nodes_pc, np.int64)
        np.add.at(a_n, tp, kl == 0)
        np.add.at(f_n, tp, kl == 1)
        np.add.at(b_n, tp, kl == 2)
        d_n = a_n + f_n + b_n
        la = np.clip((d_n + 1) // 2, a_n, a_n + f_n)   # balanced low count
        # position within node group (edges of a node are contiguous, A,flex,B)
        grp = tp
        _, first_idx, inv = np.unique(grp, return_index=True, return_inverse=True)
        cnt = np.arange(len(grp)) - first_idx[inv]
        to_a = cnt < la[tp]                            # first la edges -> pass A
        percore.append((eorder, tp, cnt, to_a, la))
        lo_cnt = np.zeros(cfg.nw, np.int64)
        hi_cnt = np.zeros(cfg.nw, np.int64)
        hb = d_n - la
        for w in range(cfg.nw):
            sl = slice(w * 128, min((w + 1) * 128, cfg.nodes_pc))
            lo_cnt[w] = max(1, la[sl].max(initial=0))
            hi_cnt[w] = max(1, hb[sl].max(initial=0))
        RA_all[ci] = lo_cnt
        RB_all[ci] = hi_cnt
    cfg.ra = [int(RA_all[:, w].max()) for w in range(cfg.nw)]
    cfg.rb = [int(RB_all[:, w].max()) for w in range(cfg.nw)]

    na = sum(cfg.ra) * 128
    nb = sum(cfg.rb) * 128
    offa = np.concatenate([[0], np.cumsum(np.array(cfg.ra) * 128)])
    offb = np.concatenate([[0], np.cumsum(np.array(cfg.rb) * 128)])

    for ci in range(c):
        eorder, tp, cnt, to_a, la = percore[ci]
        idxa = np.zeros(na, np.int64)
        idxb = np.zeros(nb, np.int64)
        maska = np.zeros(na, bool)
        maskb = np.zeros(nb, bool)
        w_ = tp // 128
        v_ = tp % 128
        r_a = cnt                       # round within A-range
        r_b = cnt - la[tp]              # round within B-range
        sa = (offa[w_] + r_a * 128 + v_)[to_a]
        sb = (offb[w_] + r_b * 128 + v_)[~to_a]
        idxa[sa] = src_row[eorder[to_a]]
        maska[sa] = True
        idxb[sb] = src_row[eorder[~to_a]] - B0
        maskb[sb] = True
        assert idxa.max() < 32768 and idxb.max() < 32768
        fslice = np.zeros((cfg.nloc, cfg.in_dim), np.float32)
        fslice[: cfg.nodes_pc] = feat[ci * cfg.nodes_pc : (ci + 1) * cfg.nodes_pc][perms[ci]]
        mk = []
        for w in range(cfg.nw):
            ma = maska[offa[w] : offa[w] + cfg.ra[w] * 128].reshape(cfg.ra[w], 128)
            mb = maskb[offb[w] : offb[w] + cfg.rb[w] * 128].reshape(cfg.rb[w], 128)
            mk.append(np.concatenate([ma, mb], 0).T.reshape(-1))  # [128 * r_w] p-major
        mask = np.concatenate(mk, 0).astype(BF16)
        in_maps.append(
            {
                "feat": fslice,
                "mask": mask,
                "idxa": wrap_idx_chunks(idxa, cfg.cb),
                "idxb": wrap_idx_chunks(idxb, cfg.cb),
            }
        )
    return in_maps, perms


def wrap_idx_chunks(idx, cb):
    n = len(idx)
    step = cb * 128
    nchunks = (n + step - 1) // step
    pad = np.zeros(nchunks * step, np.int64)
    pad[:n] = idx
    return np.stack([wrap16(pad[g * step : (g + 1) * step]) for g in range(nchunks)])


# ---------------------------------------------------------------- builder

def build(cfg: Cfg, pca_w, pca_b, mlp_w, mlp_b):
    nc = bacc.Bacc("TRN2", target_bir_lowering=False, debug=False,
                   num_devices=cfg.ncores)
    NW, D, K, DD = cfg.nw, cfg.d, cfg.k, cfg.dd
    NLOC, NFULL, B0, IN = cfg.nloc, cfg.nfull, cfg.b0, cfg.in_dim
    KC = IN // 128
    RA, RB = cfg.ra, cfg.rb
    RW = [a + b for a, b in zip(RA, RB)]
    SUMR = sum(RW)
    offa = [0]
    for a in RA:
        offa.append(offa[-1] + a * 128)
    offb = [0]
    for b in RB:
        offb.append(offb[-1] + b * 128)
    offm = [0]
    for r in RW:
        offm.append(offm[-1] + r)
    na, nb = offa[-1], offb[-1]

    feat_d = nc.declare_dram_parameter("feat", [NLOC, IN], F32, isOutput=False)
    mask_d = nc.declare_dram_parameter("mask", [SUMR * 128], BF, isOutput=False)
    na_ch = (na + cfg.cb * 128 - 1) // (cfg.cb * 128)
    nb_ch = (nb + cfg.cb * 128 - 1) // (cfg.cb * 128)
    idxa_d = nc.declare_dram_parameter("idxa", [na_ch, 128, cfg.cb * 8], I16, isOutput=False)
    idxb_d = nc.declare_dram_parameter("idxb", [nb_ch, 128, cfg.cb * 8], I16, isOutput=False)
    out_d = nc.declare_dram_parameter("out", [cfg.nodes_pc, cfg.nclass], F32, isOutput=True)

    pcaw_i = nc.inline_tensor(np.ascontiguousarray(pca_w, np.float32), name="pcaw")
    bpca_i = nc.inline_tensor(
        np.broadcast_to(np.asarray(pca_b, np.float32), (128, D)).copy(), name="bpca")
    mlpw_i = nc.inline_tensor(
        np.ascontiguousarray(mlp_w, np.float32).astype(BF16), name="mlpw")
    bmlp_i = nc.inline_tensor(
        np.broadcast_to(np.asarray(mlp_b, np.float32), (128, cfg.nclass)).copy(), name="bmlp")
    ident_i = nc.inline_tensor(np.eye(128, dtype=np.float32).astype(BF16), name="ident")
    identf_i = nc.inline_tensor(np.eye(128, dtype=np.float32), name="identf")

    xnown_d = nc.dram_tensor("xnown", [NLOC, D], BF)
    za_d = nc.dram_tensor("za", [max(na, 128), D], BF)
    zb_d = nc.dram_tensor("zb", [max(nb, 128), D], BF)
    xn_d = nc.dram_tensor("xn", [NFULL, D], BF,
                          addr_space="Shared" if cfg.ncores > 4 else "Local")
    groups = [list(range(cfg.ncores))]

    from contextlib import ExitStack
    with TileContext(nc) as tc, ExitStack() as _es:
        cpool = _es.enter_context(tc.tile_pool(name="consts", bufs=1))
        ppool = _es.enter_context(tc.tile_pool(name="persist", bufs=1))
        pool = _es.enter_context(tc.tile_pool(name="work", bufs=2))
        spool = _es.enter_context(tc.tile_pool(name="small", bufs=3))
        psum = _es.enter_context(tc.tile_pool(name="psum", bufs=2, space="PSUM"))

        ident = cpool.tile([128, 128], BF)
        nc.sync.dma_start(out=ident[:], in_=ident_i[:, :])
        identf = cpool.tile([128, 128], F32)
        nc.sync.dma_start(out=identf[:], in_=identf_i[:, :])
        bpca = cpool.tile([128, D], F32)
        nc.sync.dma_start(out=bpca[:], in_=bpca_i[:, :])
        bmlp = cpool.tile([128, cfg.nclass], F32)
        nc.sync.dma_start(out=bmlp[:], in_=bmlp_i[:, :])
        pcaw = cpool.tile([128, KC, D], F32)
        nc.sync.dma_start(out=pcaw[:], in_=pcaw_i[:, :].rearrange("(c p) d -> p c d", p=128))
        mlpw = cpool.tile([128, cfg.nclass], BF)
        nc.sync.dma_start(out=mlpw[:], in_=mlpw_i[:, :])

        c_sb = ppool.tile([128, NW * D], F32)     # [v, w*D + d] (sorted order)
        cnb_sb = ppool.tile([128, NW * D], BF)

        lib = nc.gpsimd.load_library(mlp_lib)
        first_g = [True]

        def custom_dep(gi):
            if first_g[0]:
                add_dep_helper(lib.ins, gi.ins, sync=True, reason="lib first")
                first_g[0] = False

        # ---------------- PCA: c = relu(feat @ pca_w + b)
        for w in range(NW):
            fsb = pool.tile([128, IN], F32, tag="fsb")
            nc.sync.dma_start(out=fsb[:], in_=feat_d[w * 128 : (w + 1) * 128, :])
            ftp = pool.tile([128, IN], F32, tag="ftp")
            for kc in range(KC):
                tps = psum.tile([128, 128], F32, space="PSUM", tag="tpf")
                nc.tensor.transpose(out=tps[:], in_=fsb[:, kc * 128 : (kc + 1) * 128],
                                    identity=identf[:])
                nc.scalar.copy(out=ftp[:, kc * 128 : (kc + 1) * 128], in_=tps[:])
            xps = psum.tile([128, 128], F32, space="PSUM", tag="acc")
            for kc in range(KC):
                nc.tensor.matmul(out=xps[:], lhsT=ftp[:, kc * 128 : (kc + 1) * 128],
                                 rhs=pcaw[:, kc, :], start=(kc == 0), stop=(kc == KC - 1))
            cw = c_sb[:, w * D : (w + 1) * D]
            nc.vector.tensor_tensor(out=cw, in0=xps[:], in1=bpca[:],
                                    op=mybir.AluOpType.add)
            nc.vector.tensor_scalar_max(cw, cw, 0.0)

        # ---------------- helpers
        def normalize(relu, write_xnown):
            """c <- l2norm_per_channel((relu?)(c)); cnb <- bf16(c)."""
            if relu:
                nc.vector.tensor_scalar_max(c_sb[:], c_sb[:], 0.0)
            sq = pool.tile([128, NW * D], F32, tag="nsq")
            nc.scalar.activation(sq[:], c_sb[:], mybir.ActivationFunctionType.Square)
            rn = spool.tile([128, NW * K], F32, tag="rn")
            nc.vector.tensor_reduce(
                out=rn[:], in_=sq[:].rearrange("p (g dd) -> p g dd", dd=DD),
                axis=mybir.AxisListType.X, op=mybir.AluOpType.add)
            nc.vector.tensor_scalar_max(rn[:], rn[:], 1e-24)
            nc.vector.reciprocal(rn[:], rn[:])
            nc.scalar.activation(rn[:], rn[:], mybir.ActivationFunctionType.Sqrt)
            nc.vector.tensor_tensor(
                out=c_sb[:].rearrange("p (g dd) -> p g dd", dd=DD),
                in0=c_sb[:].rearrange("p (g dd) -> p g dd", dd=DD),
                in1=rn[:, :, None].to_broadcast([128, NW * K, DD]),
                op=mybir.AluOpType.mult)
            nc.scalar.copy(out=cnb_sb[:], in_=c_sb[:])
            if write_xnown:
                nc.sync.dma_start(
                    out=xnown_d[:, :].rearrange("(w p) d -> p w d", p=128),
                    in_=cnb_sb[:].rearrange("p (w d) -> p w d", d=D))

        def zgather():
            nc.gpsimd.collective_compute(
                "AllGather", mybir.AluOpType.bypass, replica_groups=groups,
                ins=[xnown_d[:, :]], outs=[xn_d[:, :]])
            for half_i, (nch, idx_d, z_d, nrows) in enumerate(
                [(na_ch, idxa_d, za_d, na), (nb_ch, idxb_d, zb_d, nb)]):
                src_ap = xn_d[:, :] if half_i == 0 else xn_d[B0:, :]
                for g in range(nch):
                    nidx = min(cfg.cb * 128, nrows - g * cfg.cb * 128)
                    nidx = (nidx + 127) // 128 * 128
                    blocks = nidx // 128
                    it = spool.tile([128, cfg.cb * 8], I16, tag="it")
                    nc.sync.dma_start(out=it[:], in_=idx_d[g, :, :])
                    dst = pool.tile([128, cfg.cb, D], BF, tag="gdst")
                    gi = nc.gpsimd.dma_gather(
                        dst[:, :blocks, :], src_ap, it[:, : nidx // 16],
                        nidx, nidx, D, single_packet=cfg.single_packet)
                    custom_dep(gi)
                    nc.sync.dma_start(
                        out=z_d[g * cfg.cb * 128 : g * cfg.cb * 128 + nidx, :]
                        .rearrange("(b p) d -> p b d", p=128),
                        in_=dst[:, :blocks, :])

        RMAX = max(RW)

        def routing_pass():
            for w in range(NW):
                ra_w, rb_w, r_w = RA[w], RB[w], RW[w]
                zt = pool.tile([128, RMAX, D], BF, tag="zt")
                nc.sync.dma_start(
                    out=zt[:, :ra_w, :],
                    in_=za_d[offa[w] : offa[w] + ra_w * 128, :]
                    .rearrange("(b p) d -> p b d", p=128))
                nc.sync.dma_start(
                    out=zt[:, ra_w : r_w, :],
                    in_=zb_d[offb[w] : offb[w] + rb_w * 128, :]
                    .rearrange("(b p) d -> p b d", p=128))
                mk = spool.tile([128, RMAX], BF, tag="mk")
                nc.sync.dma_start(
                    out=mk[:, :r_w],
                    in_=mask_d[offm[w] * 128 : (offm[w] + r_w) * 128]
                    .rearrange("(p r) -> p r", p=128))
                cw = cnb_sb[:, w * D : (w + 1) * D]
                zc = pool.tile([128, RMAX, D], BF, tag="zc")
                nc.vector.tensor_tensor(
                    out=zc[:, :r_w, :],
                    in0=zt[:, :r_w, :],
                    in1=cw[:, None, :].to_broadcast([128, r_w, D]),
                    op=mybir.AluOpType.mult)
                p_t = spool.tile([128, RMAX * K], F32, tag="p_t")
                nc.vector.tensor_reduce(
                    out=p_t[:, : r_w * K],
                    in_=zc[:, :r_w, :].rearrange("p r (k dd) -> p (r k) dd", k=K),
                    axis=mybir.AxisListType.X, op=mybir.AluOpType.add)
                nc.scalar.activation(p_t[:, : r_w * K], p_t[:, : r_w * K],
                                     mybir.ActivationFunctionType.Exp)
                zs = spool.tile([128, RMAX], F32, tag="zs")
                nc.vector.tensor_reduce(
                    out=zs[:, :r_w],
                    in_=p_t[:, : r_w * K].rearrange("p (r k) -> p r k", k=K),
                    axis=mybir.AxisListType.X, op=mybir.AluOpType.add)
                nc.vector.reciprocal(zs[:, :r_w], zs[:, :r_w])
                rzm = spool.tile([128, RMAX], F32, tag="rzm")
                nc.vector.tensor_tensor(out=rzm[:, :r_w], in0=zs[:, :r_w],
                                        in1=mk[:, :r_w], op=mybir.AluOpType.mult)
                pn = spool.tile([128, RMAX * K], BF, tag="pn")
                nc.vector.tensor_tensor(
                    out=pn[:, : r_w * K].rearrange("p (r k) -> p r k", k=K),
                    in0=p_t[:, : r_w * K].rearrange("p (r k) -> p r k", k=K),
                    in1=rzm[:, :r_w, None].to_broadcast([128, r_w, K]),
                    op=mybir.AluOpType.mult)
                ws = pool.tile([128, RMAX, D], BF, tag="ws")
                nc.vector.tensor_tensor(
                    out=ws[:, :r_w, :].rearrange("p r (k dd) -> p (r k) dd", k=K),
                    in0=zt[:, :r_w, :].rearrange("p r (k dd) -> p (r k) dd", k=K),
                    in1=pn[:, : r_w * K, None].to_broadcast([128, r_w * K, DD]),
                    op=mybir.AluOpType.mult)
                seg = pool.tile([128, D], F32, tag="seg")
                nc.vector.tensor_reduce(
                    out=seg[:], in_=ws[:, :r_w, :].rearrange("p r d -> p d r"),
                    axis=mybir.AxisListType.X, op=mybir.AluOpType.add)
                cwf = c_sb[:, w * D : (w + 1) * D]
                nc.vector.tensor_tensor(out=cwf, in0=cwf, in1=seg[:],
                                        op=mybir.AluOpType.add)

        # ---------------- layers
        def layer_body(first_layer):
            normalize(relu=not first_layer, write_xnown=True)
            zgather()
            routing_pass()
            if cfg.unroll_t or cfg.routit <= 2:
                for _t in range(cfg.routit - 1):
                    normalize(relu=False, write_xnown=False)
                    routing_pass()
            else:
                with tc.For_i(0, cfg.routit - 1, 1) as _t:
                    normalize(relu=False, write_xnown=False)
                    routing_pass()

        for li in range(cfg.nlayer):
            layer_body(first_layer=(li == 0))

        # ---------------- head: out = log_softmax(relu(c) @ mlp_w + b)
        nc.vector.tensor_scalar_max(c_sb[:], c_sb[:], 0.0)
        nc.scalar.copy(out=cnb_sb[:], in_=c_sb[:])
        for w in range(NW):
            tps = psum.tile([128, 128], BF, space="PSUM", tag="tp")
            nc.tensor.transpose(out=tps[:], in_=cnb_sb[:, w * D : (w + 1) * D],
                                identity=ident[:])
            xT = pool.tile([128, 128], BF, tag="xT")
            nc.scalar.copy(out=xT[:], in_=tps[:])
            l2 = psum.tile([128, cfg.nclass], F32, space="PSUM", tag="l2")
            nc.tensor.matmul(out=l2[:], lhsT=xT[:], rhs=mlpw[:], start=True, stop=True)
            lg = spool.tile([128, cfg.nclass], F32, tag="lg")
            nc.vector.tensor_tensor(out=lg[:], in0=l2[:], in1=bmlp[:],
                                    op=mybir.AluOpType.add)
            nm = spool.tile([128, 1], F32, tag="nm")
            nc.vector.tensor_reduce(out=nm[:], in_=lg[:], axis=mybir.AxisListType.X,
                                    op=mybir.AluOpType.max, negate=True)
            ex = spool.tile([128, cfg.nclass], F32, tag="ex")
            nc.scalar.activation(ex[:], lg[:], mybir.ActivationFunctionType.Exp,
                                 bias=nm[:])
            se = spool.tile([128, 1], F32, tag="se")
            nc.vector.tensor_reduce(out=se[:], in_=ex[:], axis=mybir.AxisListType.X,
                                    op=mybir.AluOpType.add)
            nc.scalar.activation(se[:], se[:], mybir.ActivationFunctionType.Ln)
            nc.vector.tensor_tensor(out=se[:], in0=se[:], in1=nm[:],
                                    op=mybir.AluOpType.subtract)
            res = spool.tile([128, cfg.nclass], F32, tag="res")
            nc.vector.tensor_scalar(res[:], lg[:], se[:, :1], None,
                                    op0=mybir.AluOpType.subtract)
            rows = min(128, cfg.nodes_pc - w * 128)
            nc.sync.dma_start(out=out_d[w * 128 : w * 128 + rows, :],
                              in_=res[:rows, :])

    nc.compile()
    return nc



# ---------------------------------------------------------------- entry point

_CACHE = {}


def kernel(feat, src_trg, pca_w, pca_b, mlp_w, mlp_b):
    """Full-input DisenGCN forward on 8 NeuronCores; returns [50000, 16] f32."""
    from concourse.bass_utils import run_bass_kernel_spmd

    feat = np.asarray(feat, np.float32)
    src_trg = np.asarray(src_trg)
    cfg = Cfg(ncores=8, n_nodes=feat.shape[0], in_dim=feat.shape[1],
              d=np.asarray(pca_w).shape[1], k=8, routit=4, nlayer=3,
              nclass=np.asarray(mlp_w).shape[1])
    in_maps, perms = prep(cfg, feat, src_trg)
    key = (cfg.n_nodes, cfg.in_dim, tuple(cfg.ra), tuple(cfg.rb),
           float(np.sum(pca_w)), float(np.sum(mlp_w)))
    nc = _CACHE.get(key)
    if nc is None:
        nc = build(cfg, np.asarray(pca_w), np.asarray(pca_b),
                   np.asarray(mlp_w), np.asarray(mlp_b))
        _CACHE.clear()
        _CACHE[key] = nc
    res = run_bass_kernel_spmd(nc, in_maps, list(range(cfg.ncores)))
    outs = []
    for c in range(cfg.ncores):
        o = np.empty_like(res.results[c]["out"])
        o[perms[c]] = res.results[c]["out"]
        outs.append(o)
    return np.concatenate(outs, 0)

